# revision 38
# baseline (speedup 1.0000x reference)
"""Trainium2 Bass kernel for DifferentiableToposAttention.

Math:
  Q = sigmoid(x @ Wq.T + bq); K = sigmoid(x @ Wk.T + bk); V = x @ Wv.T + bv
  truth[q,k] = mean_d min(1 - Q[q,d] + K[k,d], 1) = 1 - (1/D) sum_d relu(Q-K)
  sum_d relu(Q[q,d]-K[k,d]) = sum_d max(Q[q,d],K[k,d]) - sum_d K[k,d]
  logit[q,k] = 10*truth = (10 + (10/D)*sumK[k]) - (10/D) * sum_d max(Q,K)
  masked (k>q) positions get logit 0 exactly (-> softmax weight exp(0)=1),
  matching the reference which fills masked scores with 0.0 before softmax.
  out[q,:] = sum_k softmax(logit)[q,k] * V[k,:]

Sharding: 8 cores; core c handles batch c//4, query tiles j=c%4 and 7-j
(128 queries each).  Tile A (=qtile j<=3) only needs keys 0..511 when the
causal mask is on; keys >=512 are all masked there, so their contribution
is the constant exp(0)*suffix-sum of V, added analytically.  Tile B
(=qtile 7-j) uses all 1024 keys.  One SPMD program, uniform shapes.

Per-core pipeline (layout: d=128 on partitions):
  - xT/KT/QT via PE transposes + matmuls, sigmoid on ACT (bias per-partition)
  - M_q[d,k] = max(KT[d,k], Q[d,q]) via DVE tensor_scalar_max (fp32 2x mode)
  - score row = ones-reduce over partitions via PE matmul; each query's row
    is scattered into PSUM partition q using a sliced stationary that has a
    single all-ones column, writing a 32-partition strip (PSUM-accumulated).
  - logits = (score * -10/128) + Cb on DVE, causal mask multiply,
    exp + row-sum on ACT (accum_out), reciprocal on DVE,
  - AV: transpose exp-weights per 128-block on PE, matmul with V natural,
    add suffix-V (tile A), scale by 1/den, DMA out.
"""

import sys

for _p in ("/opt/trn_rl_repo",):
    if _p not in sys.path:
        sys.path.insert(0, _p)

import numpy as np

import concourse.bass as bass
import concourse.mybir as mybir
import concourse.tile as tile
from concourse import bacc
from concourse.bass import ts
from concourse.masks import make_identity
from concourse.bass_utils import run_bass_kernel_spmd

F32 = mybir.dt.float32
BF16 = mybir.dt.bfloat16
FP16 = mybir.dt.float16
AF = mybir.ActivationFunctionType
ALU = mybir.AluOpType

B, S, D = 2, 1024, 128
NCORES = 8
NQT = S // 128  # 8 query tiles per batch


def _build_program(masked: bool) -> bass.Bass:
    WA = 512 if masked else 1024  # key window width for qtile A
    WB = 1024
    nc = bacc.Bacc()

    xb_d = nc.declare_dram_parameter("xb", [S, D], FP16, isOutput=False)
    xq_d = nc.declare_dram_parameter("xq", [256, D], FP16, isOutput=False)
    wqt_d = nc.declare_dram_parameter("wqt", [D, D], FP16, isOutput=False)
    wkt_d = nc.declare_dram_parameter("wkt", [D, D], FP16, isOutput=False)
    wvt_d = nc.declare_dram_parameter("wvt", [D, D], FP16, isOutput=False)
    bq_d = nc.declare_dram_parameter("bq", [D, 1], F32, isOutput=False)
    bk_d = nc.declare_dram_parameter("bk", [D, 1], F32, isOutput=False)
    bvb_d = nc.declare_dram_parameter("bvb", [D, 4 * D], F32, isOutput=False)
    qrowa_d = nc.declare_dram_parameter("qrowa", [D, 1], F32, isOutput=False)
    qrowb_d = nc.declare_dram_parameter("qrowb", [D, 1], F32, isOutput=False)
    out_d = nc.declare_dram_parameter("out", [256, D], F32, isOutput=True)

    with tile.TileContext(nc) as tc:
        with tc.tile_pool(name="singles", bufs=1) as singles:
            # ---- persistent SBUF tensors ----
            identity_bf = singles.tile([128, 128], FP16)
            make_identity(nc, identity_bf[:])
            ones_col = singles.tile([128, 1], F32)
            nc.vector.memset(ones_col[:], 1.0)
            ones_row = singles.tile([1, 128], F32)
            nc.vector.memset(ones_row[:], 1.0)
            ones_col_bf = singles.tile([128, 1], FP16)
            nc.vector.memset(ones_col_bf[:], 1.0)
            # E64: zeros except column 32 all ones. E64[:, 32-r:64-r] is a
            # [128,32] stationary whose only ones-column is local index r.
            e64 = singles.tile([128, 64], FP16)
            nc.vector.memset(e64[:], 0.0)
            nc.vector.memset(e64[:, 32:33], 1.0)
            e64n = singles.tile([128, 64], FP16)   # -1 column: negated reduce
            nc.vector.memset(e64n[:], 0.0)
            nc.vector.memset(e64n[:, 32:33], -1.0)

            xT = singles.tile([128, S], FP16)       # x^T, batch
            xqT = singles.tile([128, 256], FP16)    # x^T, this core's 256 queries
            QT = singles.tile([128, 256], F32)     # Q^T  [d, q]
            KTb = singles.tile([128, S], FP16)     # K^T in fp16 [d, k]
            Vn = singles.tile([128, NQT, 128], FP16)  # V natural [k(128), blk, e]
            Cb = singles.tile([128, S], F32)       # 10 + (10/D)*sumK[k], bcast
            svb = singles.tile([128, 128], F32)    # suffix-V bcast over q rows
            c_row = singles.tile([1, S], F32)
            sv_row = singles.tile([1, 128], F32)
            iota_r = singles.tile([128, S], F32)
            maskA = singles.tile([128, WA], F32)
            maskB = singles.tile([128, WB], F32)

            wq_sb = singles.tile([128, 128], FP16)
            wk_sb = singles.tile([128, 128], FP16)
            wv_sb = singles.tile([128, 128], FP16)
            bq_sb = singles.tile([128, 1], F32)
            bk_sb = singles.tile([128, 1], F32)
            bvb_sb = singles.tile([128, 4 * 128], F32)
            qra_sb = singles.tile([128, 1], F32)
            exp_warm = singles.tile([128, 1], F32)
            qrb_sb = singles.tile([128, 1], F32)

            nc.gpsimd.dma_start(out=wk_sb[:], in_=wkt_d[:, :])
            nc.gpsimd.dma_start(out=bk_sb[:], in_=bk_d[:, :])
            nc.gpsimd.dma_start(out=wq_sb[:], in_=wqt_d[:, :])
            nc.gpsimd.dma_start(out=bq_sb[:], in_=bq_d[:, :])
            nc.gpsimd.dma_start(out=wv_sb[:], in_=wvt_d[:, :])
            nc.gpsimd.dma_start(out=bvb_sb[:], in_=bvb_d[:, :])
            nc.gpsimd.dma_start(out=qra_sb[:], in_=qrowa_d[:, :])
            nc.gpsimd.dma_start(out=qrb_sb[:], in_=qrowb_d[:, :])

            # causal masks: mask[p, k] = 1.0 iff k <= qrow[p]
            # (iota early on gpsimd; the is_le ops are emitted inside the
            # score loop region so they don't block the first max ops)
            nc.gpsimd.iota(
                iota_r[:], pattern=[[1, S]], base=0, channel_multiplier=0,
                allow_small_or_imprecise_dtypes=True,
            )

            # ---- phase A: transposes + projections ----
            with (
                tc.tile_pool(name="ld", bufs=3) as ld,
                tc.tile_pool(name="ptr", bufs=2, space="PSUM") as ptr,
                tc.tile_pool(name="pvv", bufs=2, space="PSUM") as pvv,
                tc.tile_pool(name="prow", bufs=2, space="PSUM") as prow,
                tc.tile_pool(name="pp2", bufs=2, space="PSUM") as pp2,
            ):
                xbig = ld.tile([128, NQT, 128], FP16, tag="xbig")
                nc.sync.dma_start(
                    out=xbig[:],
                    in_=xb_d.rearrange("(t p) d -> p t d", p=128))
                xqbig = ld.tile([128, 2, 128], FP16, tag="xqbig")
                nc.sync.dma_start(
                    out=xqbig[:],
                    in_=xq_d.rearrange("(t p) d -> p t d", p=128))
                for t in range(NQT):
                    ps = ptr.tile([128, 128], FP16, tag="tr")
                    nc.tensor.transpose(ps[:], xbig[:, t, :], identity_bf[:])
                    nc.vector.tensor_copy(xT[:, ts(t, 128)], ps[:])
                for t in range(2):
                    ps = ptr.tile([128, 128], FP16, tag="tr")
                    nc.tensor.transpose(ps[:], xqbig[:, t, :], identity_bf[:])
                    nc.vector.tensor_copy(xqT[:, ts(t, 128)], ps[:])

                # K^T = (Wk^T)^T @ x^T ; sigmoid(+bk)
                for ch in range(2):
                    psk = pp2.tile([128, 512], F32, tag="proj")
                    nc.tensor.matmul(psk[:], wk_sb[:], xT[:, ts(ch, 512)])
                    nc.scalar.activation(
                        KTb[:, ts(ch, 512)], psk[:], AF.Sigmoid,
                        bias=bk_sb[:], scale=1.0)
                # Q^T for the 256 local queries
                psq = pp2.tile([128, 512], F32, tag="proj")
                nc.tensor.matmul(psq[:, 0:256], wq_sb[:], xqT[:])
                nc.scalar.activation(
                    QT[:], psq[:, 0:256], AF.Sigmoid, bias=bq_sb[:], scale=1.0)
                # preload the exp table set now (after the sigmoids)
                nc.scalar.activation(exp_warm[:], QT[:, 0:1], AF.Exp)


                # sumK row -> Cb = 10 + (10/D) * sumK  broadcast to 128 rows
                for ch in range(2):
                    pss = prow.tile([1, 512], F32, tag="rows")
                    nc.tensor.matmul(pss[:], ones_col_bf[:], KTb[:, ts(ch, 512)])
                    nc.scalar.activation(
                        c_row[:, ts(ch, 512)], pss[:], AF.Copy,
                        bias=10.0, scale=10.0 / D)
                for ch in range(2):
                    psb = pp2.tile([128, 512], F32, tag="proj")
                    nc.tensor.matmul(psb[:], ones_row[:], c_row[:, ts(ch, 512)])
                    nc.scalar.copy(Cb[:, ts(ch, 512)], psb[:])

                # V natural blocks: V[s,e] = x[s,:] @ Wv^T ; + bv (broadcast)
                for half in range(2):
                    psv = pvv.tile([128, 4, 128], F32, tag="vv")
                    for t4 in range(4):
                        t = half * 4 + t4
                        nc.tensor.matmul(psv[:, t4, :], xT[:, ts(t, 128)], wv_sb[:])
                    nc.vector.tensor_add(
                        Vn[:, ts(half, 4), :], psv[:], bvb_sb[:])
                # suffix-V (only needed when masked; zeros otherwise)
                if masked:
                    pssv = prow.tile([1, 512], F32, tag="rows")
                    for t in range(4, NQT):
                        nc.tensor.matmul(
                            pssv[:, 0:128], ones_col_bf[:], Vn[:, t, :],
                            start=(t == 4), stop=(t == NQT - 1))
                    nc.scalar.copy(sv_row[:], pssv[:, 0:128])
                    psvb = prow.tile([128, 128], F32, tag="rows")
                    nc.tensor.matmul(psvb[:], ones_row[:], sv_row[:])
                    nc.scalar.copy(svb[:], psvb[:])


            # ---- phase B: scores + softmax + AV per query tile ----
            with (
                tc.tile_pool(name="mp", bufs=10) as mp,
                tc.tile_pool(name="psc", bufs=2, space="PSUM") as psc,
                tc.tile_pool(name="po", bufs=2, space="PSUM") as po,
                tc.tile_pool(name="pw", bufs=2, space="PSUM") as pw,
                tc.tile_pool(name="sml", bufs=4) as sml,
                tc.tile_pool(name="lg", bufs=3) as lg,
                tc.tile_pool(name="wts", bufs=4) as wtsp,
                tc.tile_pool(name="ob", bufs=2) as ob,
            ):
                for W, qoff, msk, tail, is_a in [
                    (WA, 0, maskA, float(S - WA), True),
                    (WB, 128, maskB, 0.0, False),
                ]:
                    sc = psc.tile([128, 1024], F32, tag="sc")
                    # max-pass + ones-reduce row scatter; r-major order so
                    # consecutive matmuls hit different PE column strips.
                    # With the causal mask, strip s only needs the first
                    # W - 128*s keys (descending sub-tile slots); row r==0
                    # computes/writes the full group width so the strip's
                    # PSUM is fully initialized (start=True) and garbage
                    # beyond a row's slot width is finite (mask zeroes it).
                    for r in range(32):
                        for strip in range(4):
                            q = strip * 32 + r
                            # strip->slot width; strip 3 runs on ACT, so it
                            # gets the 384-wide A slot for engine balance
                            if not masked:
                                ws = W
                            elif is_a:
                                ws = (512, 256, 128, 384)[strip]
                            else:
                                ws = W - 128 * strip
                            wop = W if r == 0 else ws
                            m = mp.tile([128, 1024], FP16, tag="m")
                            qcol = QT[:, qoff + q:qoff + q + 1]
                            e_mat = e64
                            if q < 96:
                                nc.vector.tensor_scalar(
                                    m[:, 0:wop], KTb[:, 0:wop], qcol, None,
                                    ALU.max)
                            else:
                                # relu form: m = relu(Q - K); logit uses the
                                # constant 10.0 instead of Cb for these rows
                                nc.scalar.activation(
                                    m[:, 0:wop], KTb[:, 0:wop], AF.Relu,
                                    bias=qcol, scale=-1.0)
                            nch = (W if r == 0 else min(W, ws + 511)) // 512
                            for ch in range(max(1, nch)):
                                ce = W if r == 0 else ws
                                n = min(512, ce - 512 * ch)
                                nc.tensor.matmul(
                                    sc[ts(strip, 32), 512 * ch:512 * ch + n],
                                    e_mat[:, 32 - r:64 - r],
                                    m[:, 512 * ch:512 * ch + n],
                                    start=(r == 0), stop=(r == 31),
                                    skip_group_check=True,
                                    tile_position=(0, strip * 32),
                                )
                    if is_a:
                        nc.vector.tensor_scalar(
                            maskA[:], iota_r[:, 0:WA], qra_sb[:], None,
                            ALU.is_le)
                        blo = 512 if masked else 0
                        nc.vector.tensor_scalar(
                            maskB[:, blo:WB], iota_r[:, blo:WB], qrb_sb[:],
                            None, ALU.is_le)
                    # logits
                    L = lg.tile([128, 1024], F32, tag="L")
                    fs = 96
                    nc.vector.scalar_tensor_tensor(
                        out=L[0:fs, 0:W], in0=sc[0:fs, 0:W], scalar=-10.0 / D,
                        in1=Cb[0:fs, 0:W], op0=ALU.mult, op1=ALU.add)
                    nc.vector.tensor_scalar(
                        L[fs:128, 0:W], sc[fs:128, 0:W], -10.0 / D, 10.0,
                        ALU.mult, ALU.add)
                    mlo = 0 if (is_a or not masked) else 512
                    nc.vector.tensor_mul(
                        L[:, mlo:W], L[:, mlo:W], msk[:, mlo:W])
                    # exp + rowsum
                    E = lg.tile([128, 1024], FP16, tag="E")
                    rs = sml.tile([128, 1], F32, tag="rs")
                    nc.scalar.activation(
                        E[:, 0:W], L[:, 0:W], AF.Exp, accum_out=rs[:])
                    den = sml.tile([128, 1], F32, tag="den")
                    nc.vector.tensor_scalar(den[:], rs[:], tail, None, ALU.add)
                    rcp = sml.tile([128, 1], F32, tag="rcp")
                    nc.vector.reciprocal(rcp[:], den[:])
                    # AV
                    o = po.tile([128, 128], F32, tag="o")
                    nblk = W // 128
                    for t in range(nblk):
                        pwt = pw.tile([128, 128], FP16, tag="wt")
                        nc.tensor.transpose(pwt[:], E[:, ts(t, 128)], identity_bf[:])
                        wtile = wtsp.tile([128, 128], FP16, tag="wts")
                        if t % 2 == 0:
                            nc.scalar.copy(wtile[:], pwt[:])
                        else:
                            nc.vector.tensor_copy(wtile[:], pwt[:])
                        nc.tensor.matmul(
                            o[:], wtile[:], Vn[:, t, :],
                            start=(t == 0), stop=(t == nblk - 1))
                    ores = ob.tile([128, 128], F32, tag="ores")
                    if masked and is_a:
                        nc.vector.tensor_add(ores[:], o[:], svb[:])
                        nc.vector.tensor_scalar(
                            ores[:], ores[:], rcp[:], None, ALU.mult)
                    else:
                        nc.vector.tensor_scalar(
                            ores[:], o[:], rcp[:], None, ALU.mult)
                    nc.sync.dma_start(out=out_d[ts(0 if is_a else 1, 128), :], in_=ores[:])

    nc.finalize()
    return nc


_PROG_CACHE: dict[bool, bass.Bass] = {}


def _get_program(masked: bool) -> bass.Bass:
    if masked not in _PROG_CACHE:
        _PROG_CACHE[masked] = _build_program(masked)
    return _PROG_CACHE[masked]


def _core_query_rows(masked: bool, l: int) -> np.ndarray:
    """Global query indices (within the core's batch) for the 256 output
    rows, in on-device row order: group A rows 0..127, group B 128..255.

    Masked: descending width slots; strip s of group A handles 32-query
    sub-tile m = 4*(3-s)+l, group B m = 4*(7-s)+l  (m = q//32).
    Unmasked: contiguous query tiles l and 7-l.
    """
    rows = np.empty(256, dtype=np.int64)
    if masked:
        for s, wslot in enumerate((4, 2, 1, 3)):
            m = 4 * (wslot - 1) + l
            rows[32 * s:32 * s + 32] = 32 * m + np.arange(32)
        for s in range(4):
            m = 4 * (7 - s) + l
            rows[128 + 32 * s:128 + 32 * s + 32] = 32 * m + np.arange(32)
    else:
        rows[0:128] = 128 * l + np.arange(128)
        rows[128:256] = 128 * (7 - l) + np.arange(128)
    return rows


def build_in_maps(x, Wq, bq, Wk, bk, Wv, bv, masked):
    wqt = np.ascontiguousarray(Wq.T.astype(np.float16))
    wkt = np.ascontiguousarray(Wk.T.astype(np.float16))
    wvt = np.ascontiguousarray(Wv.T.astype(np.float16))
    bq2 = np.ascontiguousarray(bq.reshape(D, 1).astype(np.float32))
    bk2 = np.ascontiguousarray(bk.reshape(D, 1).astype(np.float32))
    bvb = np.ascontiguousarray(
        np.tile(bv.reshape(1, D).astype(np.float32), (D, 4)))
    in_maps = []
    for c in range(NCORES):
        b, l = divmod(c, 4)
        rows = _core_query_rows(masked, l)
        xb = np.ascontiguousarray(x[b].astype(np.float16))
        xq = np.ascontiguousarray(xb[rows])
        if masked:
            qrow = rows.astype(np.float32)
        else:
            qrow = np.full(256, 1e9, dtype=np.float32)
        in_maps.append({
            "xb": xb, "xq": xq, "wqt": wqt, "wkt": wkt, "wvt": wvt,
            "bq": bq2, "bk": bk2, "bvb": bvb,
            "qrowa": np.ascontiguousarray(qrow[0:128].reshape(D, 1)),
            "qrowb": np.ascontiguousarray(qrow[128:256].reshape(D, 1)),
        })
    return in_maps


def assemble_out(results, masked):
    out = np.empty((B, S, D), dtype=np.float32)
    for c in range(NCORES):
        b, l = divmod(c, 4)
        rows = _core_query_rows(masked, l)
        out[b, rows] = results[c]["out"]
    return out


def kernel(x, Wq, bq, Wk, bk, Wv, bv, apply_causal_mask):
    x = np.ascontiguousarray(np.asarray(x, dtype=np.float32))
    Wq = np.asarray(Wq, dtype=np.float32)
    Wk = np.asarray(Wk, dtype=np.float32)
    Wv = np.asarray(Wv, dtype=np.float32)
    bq = np.asarray(bq, dtype=np.float32)
    bk = np.asarray(bk, dtype=np.float32)
    bv = np.asarray(bv, dtype=np.float32)
    masked = bool(int(np.asarray(apply_causal_mask)))

    nc = _get_program(masked)
    in_maps = build_in_maps(x, Wq, bq, Wk, bk, Wv, bv, masked)
    res = run_bass_kernel_spmd(nc, in_maps, list(range(NCORES))).results
    return assemble_out(res, masked)


# revision 39
# speedup vs baseline: 1.0091x; 1.0091x over previous
"""Trainium2 Bass kernel for DifferentiableToposAttention.

Math:
  Q = sigmoid(x @ Wq.T + bq); K = sigmoid(x @ Wk.T + bk); V = x @ Wv.T + bv
  truth[q,k] = mean_d min(1 - Q[q,d] + K[k,d], 1) = 1 - (1/D) sum_d relu(Q-K)
  sum_d relu(Q[q,d]-K[k,d]) = sum_d max(Q[q,d],K[k,d]) - sum_d K[k,d]
  logit[q,k] = 10*truth = (10 + (10/D)*sumK[k]) - (10/D) * sum_d max(Q,K)
  masked (k>q) positions get logit 0 exactly (-> softmax weight exp(0)=1),
  matching the reference which fills masked scores with 0.0 before softmax.
  out[q,:] = sum_k softmax(logit)[q,k] * V[k,:]

Sharding: 8 cores, one SPMD program; core c handles batch c//4.  Its 256
queries are eight 32-query sub-tiles in two 128-row groups with
compile-time descending key widths (A: 512/256/128/384, B: 1024/896/768/
640); the host assigns which sub-tile fills each width slot (l = c%4), so
shapes are uniform across cores while causal-mask work is skipped.  Keys
beyond group A's 512 window are all masked there and contribute the
analytic suffix-sum of V with weight exp(0)=1.

Per-core pipeline (layout: d=128 on partitions):
  - xT/KT/QT via PE transposes + matmuls, sigmoid on ACT (bias per-partition)
  - M_q[d,k] = max(KT[d,k], Q[d,q]) fp16 on DVE (4x mode) for rows 0..95,
    relu(Q-K) on the scalar engine for rows 96..127 (engine balance)
  - score row = ones-reduce over partitions via PE matmul; each query's row
    is scattered into PSUM partition q using a sliced stationary that has a
    single all-ones column, writing a 32-partition strip (PSUM-accumulated).
  - logits = (score * -10/128) + Cb on DVE, causal mask multiply,
    exp + row-sum on ACT (accum_out), reciprocal on DVE,
  - AV: transpose exp-weights per 128-block on PE, matmul with V natural,
    add suffix-V (tile A), scale by 1/den, DMA out.
"""

import sys

for _p in ("/opt/trn_rl_repo",):
    if _p not in sys.path:
        sys.path.insert(0, _p)

import numpy as np

import concourse.bass as bass
import concourse.mybir as mybir
import concourse.tile as tile
from concourse import bacc
from concourse.bass import ts
from concourse.masks import make_identity
from concourse.bass_utils import run_bass_kernel_spmd

F32 = mybir.dt.float32
BF16 = mybir.dt.bfloat16
FP16 = mybir.dt.float16
AF = mybir.ActivationFunctionType
ALU = mybir.AluOpType

B, S, D = 2, 1024, 128
NCORES = 8
NQT = S // 128  # 8 query tiles per batch


def _build_program(masked: bool) -> bass.Bass:
    WA = 512 if masked else 1024  # key window width for qtile A
    WB = 1024
    nc = bacc.Bacc()

    xb_d = nc.declare_dram_parameter("xb", [S, D], FP16, isOutput=False)
    xq_d = nc.declare_dram_parameter("xq", [256, D], FP16, isOutput=False)
    wqt_d = nc.declare_dram_parameter("wqt", [D, D], FP16, isOutput=False)
    wkt_d = nc.declare_dram_parameter("wkt", [D, D], FP16, isOutput=False)
    wvt_d = nc.declare_dram_parameter("wvt", [D, D], FP16, isOutput=False)
    bq_d = nc.declare_dram_parameter("bq", [D, 1], F32, isOutput=False)
    bk_d = nc.declare_dram_parameter("bk", [D, 1], F32, isOutput=False)
    bvb_d = nc.declare_dram_parameter("bvb", [D, 4 * D], F32, isOutput=False)
    qrowa_d = nc.declare_dram_parameter("qrowa", [D, 1], F32, isOutput=False)
    qrowb_d = nc.declare_dram_parameter("qrowb", [D, 1], F32, isOutput=False)
    out_d = nc.declare_dram_parameter("out", [256, D], F32, isOutput=True)

    with tile.TileContext(nc) as tc:
        with tc.tile_pool(name="singles", bufs=1) as singles:
            # ---- persistent SBUF tensors ----
            identity_bf = singles.tile([128, 128], FP16)
            make_identity(nc, identity_bf[:])
            ones_col = singles.tile([128, 1], F32)
            nc.vector.memset(ones_col[:], 1.0)
            ones_row = singles.tile([1, 128], F32)
            nc.vector.memset(ones_row[:], 1.0)
            ones_col_bf = singles.tile([128, 1], FP16)
            nc.vector.memset(ones_col_bf[:], 1.0)
            # E64: zeros except column 32 all ones. E64[:, 32-r:64-r] is a
            # [128,32] stationary whose only ones-column is local index r.
            e64 = singles.tile([128, 64], FP16)
            nc.vector.memset(e64[:], 0.0)
            nc.vector.memset(e64[:, 32:33], 1.0)
            e64n = singles.tile([128, 64], FP16)   # -1 column: negated reduce
            nc.vector.memset(e64n[:], 0.0)
            nc.vector.memset(e64n[:, 32:33], -1.0)

            xT = singles.tile([128, S], FP16)       # x^T, batch
            xqT = singles.tile([128, 256], FP16)    # x^T, this core's 256 queries
            QT = singles.tile([128, 256], F32)     # Q^T  [d, q]
            KTb = singles.tile([128, S], FP16)     # K^T in fp16 [d, k]
            Vn = singles.tile([128, NQT, 128], FP16)  # V natural [k(128), blk, e]
            Cb = singles.tile([128, S], F32)       # 10 + (10/D)*sumK[k], bcast
            svb = singles.tile([128, 128], F32)    # suffix-V bcast over q rows
            c_row = singles.tile([1, S], F32)
            sv_row = singles.tile([1, 128], F32)
            iota_r = singles.tile([128, S], F32)
            maskA = singles.tile([128, WA], F32)
            maskB = singles.tile([128, WB], F32)

            wq_sb = singles.tile([128, 128], FP16)
            wk_sb = singles.tile([128, 128], FP16)
            wv_sb = singles.tile([128, 128], FP16)
            bq_sb = singles.tile([128, 1], F32)
            bk_sb = singles.tile([128, 1], F32)
            bvb_sb = singles.tile([128, 4 * 128], F32)
            qra_sb = singles.tile([128, 1], F32)
            exp_warm = singles.tile([128, 1], F32)
            qrb_sb = singles.tile([128, 1], F32)

            nc.gpsimd.dma_start(out=wk_sb[:], in_=wkt_d[:, :])
            nc.gpsimd.dma_start(out=bk_sb[:], in_=bk_d[:, :])
            nc.gpsimd.dma_start(out=wq_sb[:], in_=wqt_d[:, :])
            nc.gpsimd.dma_start(out=bq_sb[:], in_=bq_d[:, :])
            nc.gpsimd.dma_start(out=wv_sb[:], in_=wvt_d[:, :])
            nc.gpsimd.dma_start(out=bvb_sb[:], in_=bvb_d[:, :])
            nc.gpsimd.dma_start(out=qra_sb[:], in_=qrowa_d[:, :])
            nc.gpsimd.dma_start(out=qrb_sb[:], in_=qrowb_d[:, :])

            # causal masks: mask[p, k] = 1.0 iff k <= qrow[p]
            # (iota early on gpsimd; the is_le ops are emitted inside the
            # score loop region so they don't block the first max ops)
            nc.gpsimd.iota(
                iota_r[:], pattern=[[1, S]], base=0, channel_multiplier=0,
                allow_small_or_imprecise_dtypes=True,
            )

            # ---- phase A: transposes + projections ----
            with (
                tc.tile_pool(name="ld", bufs=3) as ld,
                tc.tile_pool(name="ptr", bufs=2, space="PSUM") as ptr,
                tc.tile_pool(name="pvv", bufs=2, space="PSUM") as pvv,
                tc.tile_pool(name="prow", bufs=2, space="PSUM") as prow,
                tc.tile_pool(name="pp2", bufs=2, space="PSUM") as pp2,
            ):
                xbig = ld.tile([128, NQT, 128], FP16, tag="xbig")
                nc.sync.dma_start(
                    out=xbig[:],
                    in_=xb_d.rearrange("(t p) d -> p t d", p=128))
                xqbig = ld.tile([128, 2, 128], FP16, tag="xqbig")
                nc.sync.dma_start(
                    out=xqbig[:],
                    in_=xq_d.rearrange("(t p) d -> p t d", p=128))
                for t in range(NQT):
                    ps = ptr.tile([128, 128], FP16, tag="tr")
                    nc.tensor.transpose(ps[:], xbig[:, t, :], identity_bf[:])
                    nc.vector.tensor_copy(xT[:, ts(t, 128)], ps[:])
                for t in range(2):
                    ps = ptr.tile([128, 128], FP16, tag="tr")
                    nc.tensor.transpose(ps[:], xqbig[:, t, :], identity_bf[:])
                    nc.vector.tensor_copy(xqT[:, ts(t, 128)], ps[:])

                # K^T = (Wk^T)^T @ x^T ; sigmoid(+bk)
                for ch in range(2):
                    psk = pp2.tile([128, 512], F32, tag="proj")
                    nc.tensor.matmul(psk[:], wk_sb[:], xT[:, ts(ch, 512)])
                    nc.scalar.activation(
                        KTb[:, ts(ch, 512)], psk[:], AF.Sigmoid,
                        bias=bk_sb[:], scale=1.0)
                # Q^T for the 256 local queries
                psq = pp2.tile([128, 512], F32, tag="proj")
                nc.tensor.matmul(psq[:, 0:256], wq_sb[:], xqT[:])
                nc.scalar.activation(
                    QT[:], psq[:, 0:256], AF.Sigmoid, bias=bq_sb[:], scale=1.0)
                # preload the exp table set now (after the sigmoids)
                nc.scalar.activation(exp_warm[:], QT[:, 0:1], AF.Exp)


                # sumK row -> Cb = 10 + (10/D) * sumK  broadcast to 128 rows
                for ch in range(2):
                    pss = prow.tile([1, 512], F32, tag="rows")
                    nc.tensor.matmul(pss[:], ones_col_bf[:], KTb[:, ts(ch, 512)])
                    nc.scalar.activation(
                        c_row[:, ts(ch, 512)], pss[:], AF.Copy,
                        bias=10.0, scale=10.0 / D)
                for ch in range(2):
                    psb = pp2.tile([128, 512], F32, tag="proj")
                    nc.tensor.matmul(psb[:], ones_row[:], c_row[:, ts(ch, 512)])
                    nc.scalar.copy(Cb[:, ts(ch, 512)], psb[:])

                # V natural blocks: V[s,e] = x[s,:] @ Wv^T ; + bv (broadcast)
                for half in range(2):
                    psv = pvv.tile([128, 4, 128], F32, tag="vv")
                    for t4 in range(4):
                        t = half * 4 + t4
                        nc.tensor.matmul(psv[:, t4, :], xT[:, ts(t, 128)], wv_sb[:])
                    nc.vector.tensor_add(
                        Vn[:, ts(half, 4), :], psv[:], bvb_sb[:])
                # suffix-V (only needed when masked; zeros otherwise)
                if masked:
                    pssv = prow.tile([1, 512], F32, tag="rows")
                    for t in range(4, NQT):
                        nc.tensor.matmul(
                            pssv[:, 0:128], ones_col_bf[:], Vn[:, t, :],
                            start=(t == 4), stop=(t == NQT - 1))
                    nc.scalar.copy(sv_row[:], pssv[:, 0:128])
                    psvb = prow.tile([128, 128], F32, tag="rows")
                    nc.tensor.matmul(psvb[:], ones_row[:], sv_row[:])
                    nc.scalar.copy(svb[:], psvb[:])


            # ---- phase B: scores + softmax + AV per query tile ----
            with (
                tc.tile_pool(name="mp", bufs=10) as mp,
                tc.tile_pool(name="psc", bufs=2, space="PSUM") as psc,
                tc.tile_pool(name="po", bufs=2, space="PSUM") as po,
                tc.tile_pool(name="pw", bufs=2, space="PSUM") as pw,
                tc.tile_pool(name="sml", bufs=4) as sml,
                tc.tile_pool(name="lg", bufs=3) as lg,
                tc.tile_pool(name="wts", bufs=4) as wtsp,
                tc.tile_pool(name="ob", bufs=2) as ob,
            ):
                for W, qoff, msk, tail, is_a in [
                    (WA, 0, maskA, float(S - WA), True),
                    (WB, 128, maskB, 0.0, False),
                ]:
                    sc = psc.tile([128, 1024], F32, tag="sc")
                    # max-pass + ones-reduce row scatter; r-major order so
                    # consecutive matmuls hit different PE column strips.
                    # With the causal mask, strip s only needs the first
                    # W - 128*s keys (descending sub-tile slots); row r==0
                    # computes/writes the full group width so the strip's
                    # PSUM is fully initialized (start=True) and garbage
                    # beyond a row's slot width is finite (mask zeroes it).
                    for r in range(32):
                        for strip in range(4):
                            q = strip * 32 + r
                            # strip->slot width; strip 3 runs on ACT, so it
                            # gets the 384-wide A slot for engine balance
                            if not masked:
                                ws = W
                            elif is_a:
                                ws = (512, 256, 128, 384)[strip]
                            else:
                                ws = W - 128 * strip
                            wop = W if r == 0 else ws
                            m = mp.tile([128, 1024], FP16, tag="m")
                            qcol = QT[:, qoff + q:qoff + q + 1]
                            e_mat = e64
                            if q < 96:
                                nc.vector.tensor_scalar(
                                    m[:, 0:wop], KTb[:, 0:wop], qcol, None,
                                    ALU.max)
                            else:
                                # relu form: m = relu(Q - K); logit uses the
                                # constant 10.0 instead of Cb for these rows
                                nc.scalar.activation(
                                    m[:, 0:wop], KTb[:, 0:wop], AF.Relu,
                                    bias=qcol, scale=-1.0)
                            nch = (W if r == 0 else min(W, ws + 511)) // 512
                            for ch in range(max(1, nch)):
                                ce = W if r == 0 else ws
                                n = min(512, ce - 512 * ch)
                                nc.tensor.matmul(
                                    sc[ts(strip, 32), 512 * ch:512 * ch + n],
                                    e_mat[:, 32 - r:64 - r],
                                    m[:, 512 * ch:512 * ch + n],
                                    start=(r == 0), stop=(r == 31),
                                    skip_group_check=True,
                                    tile_position=(0, strip * 32),
                                )
                    if is_a:
                        nc.vector.tensor_scalar(
                            maskA[:], iota_r[:, 0:WA], qra_sb[:], None,
                            ALU.is_le)
                        blo = 512 if masked else 0
                        nc.vector.tensor_scalar(
                            maskB[:, blo:WB], iota_r[:, blo:WB], qrb_sb[:],
                            None, ALU.is_le)
                    # logits
                    L = lg.tile([128, 1024], F32, tag="L")
                    fs = 96
                    nc.vector.scalar_tensor_tensor(
                        out=L[0:fs, 0:W], in0=sc[0:fs, 0:W], scalar=-10.0 / D,
                        in1=Cb[0:fs, 0:W], op0=ALU.mult, op1=ALU.add)
                    nc.vector.tensor_scalar(
                        L[fs:128, 0:W], sc[fs:128, 0:W], -10.0 / D, 10.0,
                        ALU.mult, ALU.add)
                    mlo = 0 if (is_a or not masked) else 512
                    nc.vector.tensor_mul(
                        L[:, mlo:W], L[:, mlo:W], msk[:, mlo:W])
                    # exp + rowsum
                    E = lg.tile([128, 1024], FP16, tag="E")
                    rs = sml.tile([128, 1], F32, tag="rs")
                    nc.scalar.activation(
                        E[:, 0:W], L[:, 0:W], AF.Exp, accum_out=rs[:])
                    den = sml.tile([128, 1], F32, tag="den")
                    nc.vector.tensor_scalar(den[:], rs[:], tail, None, ALU.add)
                    rcp = sml.tile([128, 1], F32, tag="rcp")
                    nc.vector.reciprocal(rcp[:], den[:])
                    # AV
                    o = po.tile([128, 128], F32, tag="o")
                    nblk = W // 128
                    for t in range(nblk):
                        pwt = pw.tile([128, 128], FP16, tag="wt")
                        nc.tensor.transpose(pwt[:], E[:, ts(t, 128)], identity_bf[:])
                        wtile = wtsp.tile([128, 128], FP16, tag="wts")
                        if t % 2 == 0:
                            nc.scalar.copy(wtile[:], pwt[:])
                        else:
                            nc.vector.tensor_copy(wtile[:], pwt[:])
                        nc.tensor.matmul(
                            o[:], wtile[:], Vn[:, t, :],
                            start=(t == 0), stop=(t == nblk - 1))
                    ores = ob.tile([128, 128], F32, tag="ores")
                    if masked and is_a:
                        nc.vector.tensor_add(ores[:], o[:], svb[:])
                        nc.vector.tensor_scalar(
                            ores[:], ores[:], rcp[:], None, ALU.mult)
                    else:
                        nc.vector.tensor_scalar(
                            ores[:], o[:], rcp[:], None, ALU.mult)
                    nc.sync.dma_start(out=out_d[ts(0 if is_a else 1, 128), :], in_=ores[:])

    nc.finalize()
    return nc


_PROG_CACHE: dict[bool, bass.Bass] = {}


def _get_program(masked: bool) -> bass.Bass:
    if masked not in _PROG_CACHE:
        _PROG_CACHE[masked] = _build_program(masked)
    return _PROG_CACHE[masked]


def _core_query_rows(masked: bool, l: int) -> np.ndarray:
    """Global query indices (within the core's batch) for the 256 output
    rows, in on-device row order: group A rows 0..127, group B 128..255.

    Masked: descending width slots; strip s of group A handles 32-query
    sub-tile m = 4*(3-s)+l, group B m = 4*(7-s)+l  (m = q//32).
    Unmasked: contiguous query tiles l and 7-l.
    """
    rows = np.empty(256, dtype=np.int64)
    if masked:
        for s, wslot in enumerate((4, 2, 1, 3)):
            m = 4 * (wslot - 1) + l
            rows[32 * s:32 * s + 32] = 32 * m + np.arange(32)
        for s in range(4):
            m = 4 * (7 - s) + l
            rows[128 + 32 * s:128 + 32 * s + 32] = 32 * m + np.arange(32)
    else:
        rows[0:128] = 128 * l + np.arange(128)
        rows[128:256] = 128 * (7 - l) + np.arange(128)
    return rows


def build_in_maps(x, Wq, bq, Wk, bk, Wv, bv, masked):
    wqt = np.ascontiguousarray(Wq.T.astype(np.float16))
    wkt = np.ascontiguousarray(Wk.T.astype(np.float16))
    wvt = np.ascontiguousarray(Wv.T.astype(np.float16))
    bq2 = np.ascontiguousarray(bq.reshape(D, 1).astype(np.float32))
    bk2 = np.ascontiguousarray(bk.reshape(D, 1).astype(np.float32))
    bvb = np.ascontiguousarray(
        np.tile(bv.reshape(1, D).astype(np.float32), (D, 4)))
    in_maps = []
    for c in range(NCORES):
        b, l = divmod(c, 4)
        rows = _core_query_rows(masked, l)
        xb = np.ascontiguousarray(x[b].astype(np.float16))
        xq = np.ascontiguousarray(xb[rows])
        if masked:
            qrow = rows.astype(np.float32)
        else:
            qrow = np.full(256, 1e9, dtype=np.float32)
        in_maps.append({
            "xb": xb, "xq": xq, "wqt": wqt, "wkt": wkt, "wvt": wvt,
            "bq": bq2, "bk": bk2, "bvb": bvb,
            "qrowa": np.ascontiguousarray(qrow[0:128].reshape(D, 1)),
            "qrowb": np.ascontiguousarray(qrow[128:256].reshape(D, 1)),
        })
    return in_maps


def assemble_out(results, masked):
    out = np.empty((B, S, D), dtype=np.float32)
    for c in range(NCORES):
        b, l = divmod(c, 4)
        rows = _core_query_rows(masked, l)
        out[b, rows] = results[c]["out"]
    return out


def kernel(x, Wq, bq, Wk, bk, Wv, bv, apply_causal_mask):
    x = np.ascontiguousarray(np.asarray(x, dtype=np.float32))
    Wq = np.asarray(Wq, dtype=np.float32)
    Wk = np.asarray(Wk, dtype=np.float32)
    Wv = np.asarray(Wv, dtype=np.float32)
    bq = np.asarray(bq, dtype=np.float32)
    bk = np.asarray(bk, dtype=np.float32)
    bv = np.asarray(bv, dtype=np.float32)
    masked = bool(int(np.asarray(apply_causal_mask)))

    nc = _get_program(masked)
    in_maps = build_in_maps(x, Wq, bq, Wk, bk, Wv, bv, masked)
    res = run_bass_kernel_spmd(nc, in_maps, list(range(NCORES))).results
    return assemble_out(res, masked)


# revision 40
# speedup vs baseline: 1.0215x; 1.0123x over previous
"""Trainium2 Bass kernel for DifferentiableToposAttention.

Math:
  Q = sigmoid(x @ Wq.T + bq); K = sigmoid(x @ Wk.T + bk); V = x @ Wv.T + bv
  truth[q,k] = mean_d min(1 - Q[q,d] + K[k,d], 1) = 1 - (1/D) sum_d relu(Q-K)
  sum_d relu(Q[q,d]-K[k,d]) = sum_d max(Q[q,d],K[k,d]) - sum_d K[k,d]
  logit[q,k] = 10*truth = (10 + (10/D)*sumK[k]) - (10/D) * sum_d max(Q,K)
  masked (k>q) positions get logit 0 exactly (-> softmax weight exp(0)=1),
  matching the reference which fills masked scores with 0.0 before softmax.
  out[q,:] = sum_k softmax(logit)[q,k] * V[k,:]

Sharding: 8 cores, one SPMD program; core c handles batch c//4.  Its 256
queries are eight 32-query sub-tiles in two 128-row groups with
compile-time descending key widths (A: 512/256/128/384, B: 1024/896/768/
640); the host assigns which sub-tile fills each width slot (l = c%4), so
shapes are uniform across cores while causal-mask work is skipped.  Keys
beyond group A's 512 window are all masked there and contribute the
analytic suffix-sum of V with weight exp(0)=1.

Per-core pipeline (layout: d=128 on partitions):
  - xT/KT/QT via PE transposes + matmuls, sigmoid on ACT (bias per-partition)
  - M_q[d,k] = max(KT[d,k], Q[d,q]) fp16 on DVE (4x mode) for rows 0..95,
    relu(Q-K) on the scalar engine for rows 96..127 (engine balance)
  - score row = ones-reduce over partitions via PE matmul; each query's row
    is scattered into PSUM partition q using a sliced stationary that has a
    single all-ones column, writing a 32-partition strip (PSUM-accumulated).
  - logits = (score * -10/128) + Cb on DVE, causal mask multiply,
    exp + row-sum on ACT (accum_out), reciprocal on DVE,
  - AV: transpose exp-weights per 128-block on PE, matmul with V natural,
    add suffix-V (tile A), scale by 1/den, DMA out.
"""

import sys

for _p in ("/opt/trn_rl_repo",):
    if _p not in sys.path:
        sys.path.insert(0, _p)

import numpy as np

import concourse.bass as bass
import concourse.mybir as mybir
import concourse.tile as tile
from concourse import bacc
from concourse.bass import ts
from concourse.masks import make_identity
from concourse.bass_utils import run_bass_kernel_spmd

F32 = mybir.dt.float32
BF16 = mybir.dt.bfloat16
FP16 = mybir.dt.float16
AF = mybir.ActivationFunctionType
ALU = mybir.AluOpType

B, S, D = 2, 1024, 128
NCORES = 8
NQT = S // 128  # 8 query tiles per batch


def _build_program(masked: bool) -> bass.Bass:
    WA = 512 if masked else 1024  # key window width for qtile A
    WB = 1024
    nc = bacc.Bacc()

    xb_d = nc.declare_dram_parameter("xb", [S, D], FP16, isOutput=False)
    xq_d = nc.declare_dram_parameter("xq", [256, D], FP16, isOutput=False)
    wqt_d = nc.declare_dram_parameter("wqt", [D, D], FP16, isOutput=False)
    wkt_d = nc.declare_dram_parameter("wkt", [D, D], FP16, isOutput=False)
    wvt_d = nc.declare_dram_parameter("wvt", [D, D], FP16, isOutput=False)
    bq_d = nc.declare_dram_parameter("bq", [D, 1], F32, isOutput=False)
    bk_d = nc.declare_dram_parameter("bk", [D, 1], F32, isOutput=False)
    bvb_d = nc.declare_dram_parameter("bvb", [D, 4 * D], F32, isOutput=False)
    qrowa_d = nc.declare_dram_parameter("qrowa", [D, 1], F32, isOutput=False)
    qrowb_d = nc.declare_dram_parameter("qrowb", [D, 1], F32, isOutput=False)
    out_d = nc.declare_dram_parameter("out", [256, D], F32, isOutput=True)

    with tile.TileContext(nc) as tc:
        with tc.tile_pool(name="singles", bufs=1) as singles:
            # ---- persistent SBUF tensors ----
            identity_bf = singles.tile([128, 128], FP16)
            make_identity(nc, identity_bf[:])
            ones_col = singles.tile([128, 1], F32)
            nc.vector.memset(ones_col[:], 1.0)
            ones_row = singles.tile([1, 128], F32)
            nc.vector.memset(ones_row[:], 1.0)
            ones_col_bf = singles.tile([128, 1], FP16)
            nc.vector.memset(ones_col_bf[:], 1.0)
            # E64: zeros except column 32 all ones. E64[:, 32-r:64-r] is a
            # [128,32] stationary whose only ones-column is local index r.
            e64 = singles.tile([128, 64], FP16)
            nc.vector.memset(e64[:], 0.0)
            nc.vector.memset(e64[:, 32:33], 1.0)
            e64n = singles.tile([128, 64], FP16)   # -1 column: negated reduce
            nc.vector.memset(e64n[:], 0.0)
            nc.vector.memset(e64n[:, 32:33], -1.0)

            xT = singles.tile([128, S], FP16)       # x^T, batch
            xqT = singles.tile([128, 256], FP16)    # x^T, this core's 256 queries
            QT = singles.tile([128, 256], F32)     # Q^T  [d, q]
            KTb = singles.tile([128, S], FP16)     # K^T in fp16 [d, k]
            Vn = singles.tile([128, NQT, 128], FP16)  # V natural [k(128), blk, e]
            Cb = singles.tile([128, S], F32)       # 10 + (10/D)*sumK[k], bcast
            svb = singles.tile([128, 128], F32)    # suffix-V bcast over q rows
            c_row = singles.tile([1, S], F32)
            sv_row = singles.tile([1, 128], F32)
            iota_r = singles.tile([128, S], F32)
            maskA = singles.tile([128, WA], F32)
            maskB = singles.tile([128, WB], F32)

            wq_sb = singles.tile([128, 128], FP16)
            wk_sb = singles.tile([128, 128], FP16)
            wv_sb = singles.tile([128, 128], FP16)
            bq_sb = singles.tile([128, 1], F32)
            bk_sb = singles.tile([128, 1], F32)
            bvb_sb = singles.tile([128, 4 * 128], F32)
            qra_sb = singles.tile([128, 1], F32)
            exp_warm = singles.tile([128, 1], F32)
            qrb_sb = singles.tile([128, 1], F32)

            nc.gpsimd.dma_start(out=wk_sb[:], in_=wkt_d[:, :])
            nc.gpsimd.dma_start(out=bk_sb[:], in_=bk_d[:, :])
            nc.gpsimd.dma_start(out=wq_sb[:], in_=wqt_d[:, :])
            nc.gpsimd.dma_start(out=bq_sb[:], in_=bq_d[:, :])
            nc.gpsimd.dma_start(out=wv_sb[:], in_=wvt_d[:, :])
            nc.gpsimd.dma_start(out=bvb_sb[:], in_=bvb_d[:, :])
            nc.gpsimd.dma_start(out=qra_sb[:], in_=qrowa_d[:, :])
            nc.gpsimd.dma_start(out=qrb_sb[:], in_=qrowb_d[:, :])

            # causal masks: mask[p, k] = 1.0 iff k <= qrow[p]
            # (iota early on gpsimd; the is_le ops are emitted inside the
            # score loop region so they don't block the first max ops)
            nc.gpsimd.iota(
                iota_r[:], pattern=[[1, S]], base=0, channel_multiplier=0,
                allow_small_or_imprecise_dtypes=True,
            )

            # ---- phase A: transposes + projections ----
            with (
                tc.tile_pool(name="ld", bufs=3) as ld,
                tc.tile_pool(name="ptr", bufs=2, space="PSUM") as ptr,
                tc.tile_pool(name="pvv", bufs=2, space="PSUM") as pvv,
                tc.tile_pool(name="prow", bufs=2, space="PSUM") as prow,
                tc.tile_pool(name="pp2", bufs=2, space="PSUM") as pp2,
            ):
                xbig = ld.tile([128, NQT, 128], FP16, tag="xbig")
                nc.sync.dma_start(
                    out=xbig[:],
                    in_=xb_d.rearrange("(t p) d -> p t d", p=128))
                xqbig = ld.tile([128, 2, 128], FP16, tag="xqbig")
                nc.sync.dma_start(
                    out=xqbig[:],
                    in_=xq_d.rearrange("(t p) d -> p t d", p=128))
                for t in range(NQT):
                    ps = ptr.tile([128, 128], FP16, tag="tr")
                    nc.tensor.transpose(ps[:], xbig[:, t, :], identity_bf[:])
                    nc.vector.tensor_copy(xT[:, ts(t, 128)], ps[:])
                for t in range(2):
                    ps = ptr.tile([128, 128], FP16, tag="tr")
                    nc.tensor.transpose(ps[:], xqbig[:, t, :], identity_bf[:])
                    nc.vector.tensor_copy(xqT[:, ts(t, 128)], ps[:])

                # K^T = (Wk^T)^T @ x^T ; sigmoid(+bk)
                for ch in range(2):
                    psk = pp2.tile([128, 512], F32, tag="proj")
                    nc.tensor.matmul(psk[:], wk_sb[:], xT[:, ts(ch, 512)])
                    nc.scalar.activation(
                        KTb[:, ts(ch, 512)], psk[:], AF.Sigmoid,
                        bias=bk_sb[:], scale=1.0)
                # Q^T for the 256 local queries
                psq = pp2.tile([128, 512], F32, tag="proj")
                nc.tensor.matmul(psq[:, 0:256], wq_sb[:], xqT[:])
                nc.scalar.activation(
                    QT[:], psq[:, 0:256], AF.Sigmoid, bias=bq_sb[:], scale=1.0)
                # preload the exp table set now (after the sigmoids)
                nc.scalar.activation(exp_warm[:], QT[:, 0:1], AF.Exp)


                # sumK row -> Cb = 10 + (10/D) * sumK  broadcast to 128 rows
                for ch in range(2):
                    pss = prow.tile([1, 512], F32, tag="rows")
                    nc.tensor.matmul(pss[:], ones_col_bf[:], KTb[:, ts(ch, 512)])
                    nc.scalar.activation(
                        c_row[:, ts(ch, 512)], pss[:], AF.Copy,
                        bias=10.0, scale=10.0 / D)
                for ch in range(2):
                    psb = pp2.tile([128, 512], F32, tag="proj")
                    nc.tensor.matmul(psb[:], ones_row[:], c_row[:, ts(ch, 512)])
                    nc.scalar.copy(Cb[:, ts(ch, 512)], psb[:])

                # V natural blocks: V[s,e] = x[s,:] @ Wv^T ; + bv (broadcast)
                for half in range(2):
                    psv = pvv.tile([128, 4, 128], F32, tag="vv")
                    for t4 in range(4):
                        t = half * 4 + t4
                        nc.tensor.matmul(psv[:, t4, :], xT[:, ts(t, 128)], wv_sb[:])
                    nc.vector.tensor_add(
                        Vn[:, ts(half, 4), :], psv[:], bvb_sb[:])
                # suffix-V (only needed when masked; zeros otherwise)
                if masked:
                    pssv = prow.tile([1, 512], F32, tag="rows")
                    for t in range(4, NQT):
                        nc.tensor.matmul(
                            pssv[:, 0:128], ones_col_bf[:], Vn[:, t, :],
                            start=(t == 4), stop=(t == NQT - 1))
                    nc.scalar.copy(sv_row[:], pssv[:, 0:128])
                    psvb = prow.tile([128, 128], F32, tag="rows")
                    nc.tensor.matmul(psvb[:], ones_row[:], sv_row[:])
                    nc.scalar.copy(svb[:], psvb[:])


            # ---- phase B: scores + softmax + AV per query tile ----
            with (
                tc.tile_pool(name="mp", bufs=10) as mp,
                tc.tile_pool(name="psc", bufs=2, space="PSUM") as psc,
                tc.tile_pool(name="po", bufs=2, space="PSUM") as po,
                tc.tile_pool(name="pw", bufs=2, space="PSUM") as pw,
                tc.tile_pool(name="sml", bufs=4) as sml,
                tc.tile_pool(name="lg", bufs=3) as lg,
                tc.tile_pool(name="wts", bufs=4) as wtsp,
                tc.tile_pool(name="ob", bufs=2) as ob,
            ):
                for W, qoff, msk, tail, is_a in [
                    (WA, 0, maskA, float(S - WA), True),
                    (WB, 128, maskB, 0.0, False),
                ]:
                    sc = psc.tile([128, 1024], F32, tag="sc")
                    # max-pass + ones-reduce row scatter; r-major order so
                    # consecutive matmuls hit different PE column strips.
                    # With the causal mask, strip s only needs the first
                    # W - 128*s keys (descending sub-tile slots); row r==0
                    # computes/writes the full group width so the strip's
                    # PSUM is fully initialized (start=True) and garbage
                    # beyond a row's slot width is finite (mask zeroes it).
                    for r in range(32):
                        for strip in range(4):
                            q = strip * 32 + r
                            # strip->slot width; strip 3 runs on ACT, so it
                            # gets the 384-wide A slot for engine balance
                            if not masked:
                                ws = W
                            elif is_a:
                                ws = (384, 256, 128, 512)[strip]
                            else:
                                ws = (1024, 896, 640, 768)[strip]
                            wop = W if r == 0 else ws
                            m = mp.tile([128, 1024], FP16, tag="m")
                            qcol = QT[:, qoff + q:qoff + q + 1]
                            e_mat = e64
                            if q < 96:
                                nc.vector.tensor_scalar(
                                    m[:, 0:wop], KTb[:, 0:wop], qcol, None,
                                    ALU.max)
                            else:
                                # relu form: m = relu(Q - K); logit uses the
                                # constant 10.0 instead of Cb for these rows
                                nc.scalar.activation(
                                    m[:, 0:wop], KTb[:, 0:wop], AF.Relu,
                                    bias=qcol, scale=-1.0)
                            nch = (W if r == 0 else min(W, ws + 511)) // 512
                            for ch in range(max(1, nch)):
                                ce = W if r == 0 else ws
                                n = min(512, ce - 512 * ch)
                                nc.tensor.matmul(
                                    sc[ts(strip, 32), 512 * ch:512 * ch + n],
                                    e_mat[:, 32 - r:64 - r],
                                    m[:, 512 * ch:512 * ch + n],
                                    start=(r == 0), stop=(r == 31),
                                    skip_group_check=True,
                                    tile_position=(0, strip * 32),
                                )
                    if is_a:
                        nc.vector.tensor_scalar(
                            maskA[:], iota_r[:, 0:WA], qra_sb[:], None,
                            ALU.is_le)
                        blo = 512 if masked else 0
                        nc.vector.tensor_scalar(
                            maskB[:, blo:WB], iota_r[:, blo:WB], qrb_sb[:],
                            None, ALU.is_le)
                    # logits
                    L = lg.tile([128, 1024], F32, tag="L")
                    fs = 96
                    nc.vector.scalar_tensor_tensor(
                        out=L[0:fs, 0:W], in0=sc[0:fs, 0:W], scalar=-10.0 / D,
                        in1=Cb[0:fs, 0:W], op0=ALU.mult, op1=ALU.add)
                    nc.vector.tensor_scalar(
                        L[fs:128, 0:W], sc[fs:128, 0:W], -10.0 / D, 10.0,
                        ALU.mult, ALU.add)
                    mlo = 0 if (is_a or not masked) else 512
                    nc.vector.tensor_mul(
                        L[:, mlo:W], L[:, mlo:W], msk[:, mlo:W])
                    # exp + rowsum
                    E = lg.tile([128, 1024], FP16, tag="E")
                    rs = sml.tile([128, 1], F32, tag="rs")
                    nc.scalar.activation(
                        E[:, 0:W], L[:, 0:W], AF.Exp, accum_out=rs[:])
                    den = sml.tile([128, 1], F32, tag="den")
                    nc.vector.tensor_scalar(den[:], rs[:], tail, None, ALU.add)
                    rcp = sml.tile([128, 1], F32, tag="rcp")
                    nc.vector.reciprocal(rcp[:], den[:])
                    # AV
                    o = po.tile([128, 128], F32, tag="o")
                    nblk = W // 128
                    for t in range(nblk):
                        pwt = pw.tile([128, 128], FP16, tag="wt")
                        nc.tensor.transpose(pwt[:], E[:, ts(t, 128)], identity_bf[:])
                        wtile = wtsp.tile([128, 128], FP16, tag="wts")
                        if t % 2 == 0:
                            nc.scalar.copy(wtile[:], pwt[:])
                        else:
                            nc.vector.tensor_copy(wtile[:], pwt[:])
                        nc.tensor.matmul(
                            o[:], wtile[:], Vn[:, t, :],
                            start=(t == 0), stop=(t == nblk - 1))
                    ores = ob.tile([128, 128], F32, tag="ores")
                    if masked and is_a:
                        nc.vector.tensor_add(ores[:], o[:], svb[:])
                        nc.vector.tensor_scalar(
                            ores[:], ores[:], rcp[:], None, ALU.mult)
                    else:
                        nc.vector.tensor_scalar(
                            ores[:], o[:], rcp[:], None, ALU.mult)
                    nc.sync.dma_start(out=out_d[ts(0 if is_a else 1, 128), :], in_=ores[:])

    nc.finalize()
    return nc


_PROG_CACHE: dict[bool, bass.Bass] = {}


def _get_program(masked: bool) -> bass.Bass:
    if masked not in _PROG_CACHE:
        _PROG_CACHE[masked] = _build_program(masked)
    return _PROG_CACHE[masked]


def _core_query_rows(masked: bool, l: int) -> np.ndarray:
    """Global query indices (within the core's batch) for the 256 output
    rows, in on-device row order: group A rows 0..127, group B 128..255.

    Masked: descending width slots; strip s of group A handles 32-query
    sub-tile m = 4*(3-s)+l, group B m = 4*(7-s)+l  (m = q//32).
    Unmasked: contiguous query tiles l and 7-l.
    """
    rows = np.empty(256, dtype=np.int64)
    if masked:
        for s, wslot in enumerate((3, 2, 1, 4)):
            m = 4 * (wslot - 1) + l
            rows[32 * s:32 * s + 32] = 32 * m + np.arange(32)
        for s, wslot in enumerate((8, 7, 5, 6)):
            m = 4 * (wslot - 1) + l
            rows[128 + 32 * s:128 + 32 * s + 32] = 32 * m + np.arange(32)
    else:
        rows[0:128] = 128 * l + np.arange(128)
        rows[128:256] = 128 * (7 - l) + np.arange(128)
    return rows


def build_in_maps(x, Wq, bq, Wk, bk, Wv, bv, masked):
    wqt = np.ascontiguousarray(Wq.T.astype(np.float16))
    wkt = np.ascontiguousarray(Wk.T.astype(np.float16))
    wvt = np.ascontiguousarray(Wv.T.astype(np.float16))
    bq2 = np.ascontiguousarray(bq.reshape(D, 1).astype(np.float32))
    bk2 = np.ascontiguousarray(bk.reshape(D, 1).astype(np.float32))
    bvb = np.ascontiguousarray(
        np.tile(bv.reshape(1, D).astype(np.float32), (D, 4)))
    in_maps = []
    for c in range(NCORES):
        b, l = divmod(c, 4)
        rows = _core_query_rows(masked, l)
        xb = np.ascontiguousarray(x[b].astype(np.float16))
        xq = np.ascontiguousarray(xb[rows])
        if masked:
            qrow = rows.astype(np.float32)
        else:
            qrow = np.full(256, 1e9, dtype=np.float32)
        in_maps.append({
            "xb": xb, "xq": xq, "wqt": wqt, "wkt": wkt, "wvt": wvt,
            "bq": bq2, "bk": bk2, "bvb": bvb,
            "qrowa": np.ascontiguousarray(qrow[0:128].reshape(D, 1)),
            "qrowb": np.ascontiguousarray(qrow[128:256].reshape(D, 1)),
        })
    return in_maps


def assemble_out(results, masked):
    out = np.empty((B, S, D), dtype=np.float32)
    for c in range(NCORES):
        b, l = divmod(c, 4)
        rows = _core_query_rows(masked, l)
        out[b, rows] = results[c]["out"]
    return out


def kernel(x, Wq, bq, Wk, bk, Wv, bv, apply_causal_mask):
    x = np.ascontiguousarray(np.asarray(x, dtype=np.float32))
    Wq = np.asarray(Wq, dtype=np.float32)
    Wk = np.asarray(Wk, dtype=np.float32)
    Wv = np.asarray(Wv, dtype=np.float32)
    bq = np.asarray(bq, dtype=np.float32)
    bk = np.asarray(bk, dtype=np.float32)
    bv = np.asarray(bv, dtype=np.float32)
    masked = bool(int(np.asarray(apply_causal_mask)))

    nc = _get_program(masked)
    in_maps = build_in_maps(x, Wq, bq, Wk, bk, Wv, bv, masked)
    res = run_bass_kernel_spmd(nc, in_maps, list(range(NCORES))).results
    return assemble_out(res, masked)
